# revision 40
# baseline (speedup 1.0000x reference)
"""FlowNetC correlation layer on 8 Trainium2 NeuronCores.

Math: out[b, d, y, x] = (1/256) * sum_c in1[b,c,y,x] * in2pad[b,c,y+dy,x+dx]
with (dy, dx) on a 21x21 stride-2 grid spanning [-20, 20], zero padding 20.

Strategy (per core = one batch sample; batch is exactly 8):
- Displacements have stride 2, so the problem splits into 4 independent parity
  classes. Each class: in1c [256, 32, 48] against a padded in2c [256, 52, 68]
  with stride-1 displacements dy', dx' in [0, 20].
- Gram band matmuls: for each class and group of 4 subsampled x-columns, run 4
  col-tiled matmuls (M=32 each, tile_position=(0, 32*xg)). Tile xg's stationary
  is in1c[:, :, x0] (32 ys columns); its moving tensor is the 21-wide window
  in2c[:, :, x0:x0+21] over all 52 rows (N = 52*21 = 1092, split into PSUM-bank
  chunks of 504/504/84). PSUM partition p = 32*xg + ys then holds the full
  441-displacement vector contiguously at columns [21*ys, 21*ys + 441).
- The per-partition shear (21*ys) is undone by SBUF->SBUF DMAs (8 ys per DMA
  via a partition+offset compound stride) at xsg-triplet granularity, spread
  over the sync/gpsimd/scalar queues, writing dense [pixel, d] tiles with
  partition order p = 4*ys + xg.
- TensorE transposes flip dense [pixel, d] tiles to [d, pixel]; these are
  software-pipelined INTO the next class's matmul stream (the PE queue is
  in-order, so transpose groups are interleaved between matmul groups two
  slots after their de-shear DMAs were issued). Scalar/vector evictions
  scatter pt tiles into a d-major bf16 assembly buffer; per-d-chunk output
  DMAs fire right after each chunk's final scatter (bf16 HBM out, host casts).
- Matmul inputs are bf16; the 1/256 normalization is folded into in1's bf16
  cast exactly (exponent shift).
"""

import os
import sys

for _p in ("/opt/trn_rl_repo", "/root/.axon_site/_ro/trn_rl_repo"):
    if os.path.isdir(_p) and _p not in sys.path:
        sys.path.insert(0, _p)

from contextlib import ExitStack

import ml_dtypes
import numpy as np

import concourse.bacc as bacc
import concourse.bass as bass
import concourse.mybir as mybir
import concourse.tile as tile
from concourse.bass_utils import run_bass_kernel_spmd
from concourse.masks import make_identity

B, C, H, W = 8, 256, 64, 96
NYS, NXS = 32, 48          # subsampled class grid
RB, CB = 52, 68            # padded class grid (rows/cols)
ND = 441                   # displacements
WB = 1092                  # band width per xs-column (52 rows * 21 dx)
NG = 12                    # xs-column groups per class band
FB = NG * WB               # class band free size
NPIX = H * W               # 6144
NDP = 441                  # dense per-xsg pitch
DCHUNKS = [(0, 128), (128, 128), (256, 128), (384, 57)]
# Padded in2 rows 0-9 and 42-51 are all-zero (pad 10 in subsampled space), so
# the gram only computes rows [10, 42); the zero band regions are memset once.
GRAM_CHUNKS = [(10, 34), (34, 42)]  # ysB row ranges per PSUM bank

F32 = mybir.dt.float32
BF16 = mybir.dt.bfloat16


def build(reps=1, band_bf16=True, gram_only=False):
    """reps>1 repeats the whole pipeline in-NEFF (timing: cancels fixed overhead).
    gram_only: skip shear/transpose/assembly (hardware experiment for PE time)."""
    BDT = BF16 if band_bf16 else F32
    nc = bacc.Bacc("TRN2", target_bir_lowering=False, debug=False, num_devices=8)
    in1p = nc.declare_dram_parameter("in1p", [4, 128, 2, NXS, NYS], BF16, isOutput=False)
    in2p = nc.declare_dram_parameter("in2p", [4, 128, 2, RB, CB], BF16, isOutput=False)
    outp = nc.declare_dram_parameter("out", [ND, H, W], BF16, isOutput=True)

    with tile.TileContext(nc) as tc:
        with ExitStack() as ctx:
            const_pool = ctx.enter_context(tc.tile_pool(name="const", bufs=1))
            in2_pool = ctx.enter_context(tc.tile_pool(name="in2", bufs=2))
            band_pool = ctx.enter_context(tc.tile_pool(name="band", bufs=2))
            dense_pool = ctx.enter_context(tc.tile_pool(name="dense", bufs=2))
            out_pool = ctx.enter_context(tc.tile_pool(name="outsb", bufs=1))
            pg_pool = ctx.enter_context(tc.tile_pool(name="pg", bufs=2, space="PSUM"))
            pt_pool = ctx.enter_context(tc.tile_pool(name="pt", bufs=4, space="PSUM"))

            ident = const_pool.tile([128, 128], BDT)
            make_identity(nc, ident)

            # resident in1: [c, cls, k, xs, ys], loaded per class (contiguous
            # 6 KB runs) so the first matmul only waits for its own class.
            in1_sb = const_pool.tile([128, 4, 2, NXS, NYS], BF16)

            def load_in1(cid):
                cs = 2 * NXS * NYS
                nc.sync.dma_start(
                    out=bass.AP(in1_sb.tensor, in1_sb.offset + cid * cs,
                                [[4 * cs, 128], [1, cs]]),
                    in_=bass.AP(in1p, cid * 128 * cs, [[cs, 128], [1, cs]]),
                )

            # persistent d-major assembly buffers, one per d-chunk
            out_sb = [out_pool.tile([128, NPIX], BF16, tag=f"out{dc}", name=f"out_sb{dc}")
                      for dc in range(4)]

            def load_in2(cid):
                """Two DMAs (one per k chunk) so the first matmuls only wait
                for the k=0 half; class 0's k=0 is split across both HW
                queues (head critical path)."""
                in2_sb = in2_pool.tile([128, 2, RB, CB], BF16)
                for k in range(2):
                    nc.scalar.dma_start(
                        out=bass.AP(in2_sb.tensor, in2_sb.offset + k * RB * CB,
                                    [[2 * RB * CB, 128], [1, RB * CB]]),
                        in_=bass.AP(in2p, cid * 2 * 128 * RB * CB + k * RB * CB,
                                    [[2 * RB * CB, 128], [1, RB * CB]]),
                    )
                return in2_sb

            def mm_group(cid, in2_sb, band, xsg):
                """Matmuls + PSUM evictions for one xsg group (nonzero rows
                [10, 42) only)."""
                pg = pg_pool.tile([128, 2, 512], F32)
                # ch outer, xg inner: adjacent matmuls hit different PE
                # column groups so their moving streams overlap.
                for ch, (r0, r1) in enumerate(GRAM_CHUNKS):
                    ncols = (r1 - r0) * 21
                    for k in range(2):
                        for xg in range(4):
                            x0 = 4 * xsg + xg
                            lhsT = in1_sb[:, cid, k, x0, :]
                            rhs = in2_sb[:, k, r0:r1, x0:x0 + 21]
                            nc.tensor.matmul(
                                pg[32 * xg:32 * (xg + 1), ch, 0:ncols],
                                lhsT, rhs,
                                start=(k == 0), stop=(k == 1),
                                tile_position=(0, 32 * xg),
                            )
                base = xsg * WB
                nc.vector.tensor_copy(
                    out=band[:, base + 210:base + 714],
                    in_=pg[:, 0, 0:504],
                )
                nc.vector.tensor_copy(
                    out=band[:, base + 714:base + 882],
                    in_=pg[:, 1, 0:168],
                )

            DS_ENGS = [nc.sync, nc.gpsimd, nc.scalar]

            def deshear_h(cid, band, dense, h):
                """De-shear DMAs for xsg half h (6 xsg groups): one DMA per
                ys (441-element runs; used for the last class so the tail's
                transposes can start at half-class granularity)."""
                for ys in range(NYS):
                    src = bass.AP(band.tensor,
                                  band.offset + ys * (FB + 21) + 6 * h * WB,
                                  [[32 * FB, 4], [WB, 6], [1, ND]])
                    dst = bass.AP(dense.tensor,
                                  dense.offset + 4 * ys * (NG * NDP) + 6 * h * NDP,
                                  [[NG * NDP, 4], [NDP, 6], [1, ND]])
                    DS_ENGS[ys % 3].dma_start(out=dst, in_=src)

            def tp_group(cid, dense, s, dc, fire):
                """Transpose xsg-triplet s x d-chunk dc to d-major and
                scatter into the assembly buffer; optionally fire the
                output DMA for dc (last class, last s only). `dense` is
                either a WB-pitch draw tile (classes 0-2) or an NDP-pitch
                dense tile (class 3)."""
                d0, dcw = DCHUNKS[dc]
                pitch, ptot = NDP, NG * NDP
                py, px = cid // 2, cid % 2
                pt = pt_pool.tile([128, 384], BDT)
                for j in range(3):
                    off = (3 * s + j) * pitch + d0
                    nc.tensor.transpose(
                        pt[0:dcw, j * 128:(j + 1) * 128],
                        bass.AP(dense.tensor, dense.offset + off,
                                [[ptot, 128], [1, dcw]]),
                        ident[:],
                    )
                ob = out_sb[dc]
                src = bass.AP(pt.tensor, pt.offset,
                              [[384, dcw], [4, 32], [128, 3], [1, 4]])
                doff = 96 * py + px + 24 * s
                dst = bass.AP(ob.tensor, ob.offset + doff,
                              [[NPIX, dcw], [192, 32], [8, 3], [2, 4]])
                if (s + dc) % 2 == 0:
                    nc.scalar.copy(out=dst, in_=src)
                else:
                    nc.vector.tensor_copy(out=dst, in_=src)
                if fire:
                    fire_out_dma(dc)

            FIRE_ENGS = None

            def fire_out_dma(dc):
                d0, dcw = DCHUNKS[dc]
                ob = out_sb[dc]
                eng = (FIRE_ENGS or [nc.sync, nc.scalar, nc.sync, nc.scalar])[dc]
                eng.dma_start(
                    out=bass.AP(outp, d0 * NPIX, [[NPIX, dcw], [1, NPIX]]),
                    in_=bass.AP(ob.tensor, ob.offset, [[NPIX, dcw], [1, NPIX]]),
                )

            for rep in range(reps):
                # pending transpose groups: (ready_slot, cid, dense, s, dc, fire)
                pending = []
                slot = 0
                in2_cur = None
                for cid in range(4):
                    if rep == 0:
                        load_in1(cid)
                    if cid == 0:
                        in2_cur = load_in2(0)
                    in2_next = load_in2(cid + 1) if cid < 3 else None
                    band = band_pool.tile([128, FB], BDT)
                    last = cid == 3
                    dense = dense_pool.tile([128, NG, NDP], BDT, name="dense")
                    if rep == 0 and cid < 2:
                        # zero the never-computed band regions (in2 pad rows
                        # 0-9 and 42-51) once per physical ring buffer
                        nc.gpsimd.memset(
                            bass.AP(band.tensor, band.offset,
                                    [[FB, 128], [WB, NG], [882, 2], [1, 210]]),
                            0.0,
                        )
                    for xsg in range(NG):
                        mm_group(cid, in2_cur, band, xsg)
                        if not gram_only and xsg % 6 == 5:
                            h = xsg // 6
                            deshear_h(cid, band, dense, h)
                            for s in (2 * h, 2 * h + 1):
                                for dc in range(4):
                                    pending.append(
                                        (slot + 2, cid, dense, s, dc,
                                         last and s == 3))
                        if not gram_only:
                            drained = 0
                            while pending and drained < 2 and pending[0][0] <= slot:
                                _, pcid, pdense, ps, pdc, pfire = pending.pop(0)
                                tp_group(pcid, pdense, ps, pdc, pfire)
                                drained += 1
                        slot += 1
                    in2_cur = in2_next
                if gram_only:
                    nc.vector.tensor_copy(out=out_sb[0][:, :512], in_=band[:, :512])
                    nc.sync.dma_start(
                        out=bass.AP(outp, 0, [[NPIX, 128], [1, NPIX]]),
                        in_=bass.AP(out_sb[0].tensor, out_sb[0].offset,
                                    [[NPIX, 128], [1, NPIX]]),
                    )
                    continue
                # final flush, dc-major so each d-chunk's output DMA fires as
                # early as possible and its wire time overlaps the remaining
                # transposes/scatters
                pending.sort(key=lambda e: (e[1], e[4], e[3]))
                for _, pcid, pdense, ps, pdc, pfire in pending:
                    tp_group(pcid, pdense, ps, pdc, pfire)

    nc.compile()
    return nc


def prep_inputs(input1, input2):
    """Host-side: parity split, pad, bf16 cast, fold 1/256 into in1."""
    in_maps = []
    for b in range(B):
        a1 = (input1[b].astype(np.float32) / 256.0).reshape(2, 128, H, W)
        a2 = input2[b].astype(np.float32).reshape(2, 128, H, W)
        in1p = np.empty((4, 128, 2, NXS, NYS), dtype=ml_dtypes.bfloat16)
        in2p = np.zeros((4, 128, 2, RB, CB), dtype=ml_dtypes.bfloat16)
        for cid in range(4):
            py, px = cid // 2, cid % 2
            in1p[cid] = a1[:, :, py::2, px::2].transpose(1, 0, 3, 2).astype(ml_dtypes.bfloat16)
            in2p[cid, :, :, 10:42, 10:58] = a2[:, :, py::2, px::2].transpose(1, 0, 2, 3).astype(ml_dtypes.bfloat16)
        in_maps.append({"in1p": in1p, "in2p": in2p})
    return in_maps


_NC = None


def get_nc():
    global _NC
    if _NC is None:
        _NC = build()
    return _NC


def kernel(input1, input2):
    nc = get_nc()
    in_maps = prep_inputs(np.asarray(input1), np.asarray(input2))
    r = run_bass_kernel_spmd(nc, in_maps, core_ids=list(range(8)))
    return np.stack([r.results[i]["out"] for i in range(B)]).astype(np.float32)
